# revision 17
# baseline (speedup 1.0000x reference)
"""Trainium2 Bass kernel for nn_L1OutUB (L1-out upper bound contrastive loss).

Math: the reference builds a [B,B,B] tensor `inpt[a,i,j] = all_probs[i,j] +
(-20 if a==i else 0)` and logsumexps over `a`.  That logsumexp is exactly
`all_probs[i,j] + log(B-1+e^-20)`, so

    result = mean(positive) - mean(all_probs) - log1p(e^-20 / (B-1))

and `sum_j all_probs[i,j]` collapses onto per-column moments of y
(S2[d] = sum_j y[j,d]^2, M1[d] = sum_j y[j,d]).  The -0.5*logvar terms
cancel exactly between positive and negative, and the per-(i,d) mu^2 terms
cancel between the positive and all-pairs branches:

    contrib[i,d] = inv[i,d] * ( mu[i,d]*(yc/B - M1/B^2) + K[i,d] )
      K   = S2/(2B^2) - yc^2/(2B)     (yc = matched y rows, feature-major)
      inv = exp(-tanh(z_lv))

Sharding: rows of x across 8 cores (64 rows each); every core gets the full
y (column-rotated so its matched rows sit at cols 0:64 of yT) and computes
the global column moments redundantly.  Host sums the 8 scalar partials
(the "all-reduce").

Layout/overlap decisions (all transposes done on host; PE does matmuls only):
  - Two input DMAs per HWDGE queue: blob1 = [w1|b1|xT chunks 0:2|yT half A],
    blob2 = [w2|xT chunks 3:5|yT half B].  x parts stream ahead of y parts;
    y moments are computed per-half as the data lands.
  - y is shipped pre-transposed (yT [128,512]) so moments are free-dim DVE
    reductions and yc/yc^2 are column slices.
  - L1 runs both nets in one 6-matmul chain ([128,41] stationaries, mu rows
    0:8, lv rows 32:40).  Bias+relu fused into one ACT op whose bias column
    also manufactures the two all-ones rows (bias[8]=bias[40]=1, relu(0+1)).
  - L2 folds its biases via those ones-rows, so mu / z_lv leave PSUM done.
  - ACT does relu/tanh/exp only (one table set, load overlaps the DMAs).
  - Final reduce: free-dim DVE reduce -> [128,1], PE matmul against a ones
    column -> [1,1] -> single 4-byte output DMA (a [128,1] output DMA costs
    ~7us in scattered-write completion; don't do that).
"""

import numpy as np

import concourse.bacc as bacc
import concourse.tile as tile
from concourse import mybir

F32 = mybir.dt.float32
AF = mybir.ActivationFunctionType
ALU = mybir.AluOpType

B, X_DIM, Y_DIM, HID = 512, 768, 128, 8
N_CORES = 8
R = B // N_CORES          # rows per core = 64
XC = X_DIM // 128         # x feature chunks = 6
YH = B // 2               # yT half width = 256

W1C = 41                  # L1 stationary cols (mu 0:8, lv 32:40, 40 = ones)
A_W1 = XC * W1C           # 246
A_B1 = A_W1               # bias column index
A_XT = A_B1 + 1           # 247
B1_COLS = A_XT + 3 * R    # blob1 width: 439 (w1|b1|xT chunks 0:2)
B_W2 = Y_DIM              # blob2: w2 block cols 0:128
B_XT = B_W2               # xT chunks 3:5 at 128:320
B2_COLS = B_XT + 3 * R    # blob2 width: 320

_CACHE = {}


def _build():
    nc = bacc.Bacc("TRN2", target_bir_lowering=False, debug=False,
                   num_devices=N_CORES)

    b1_d = nc.dram_tensor("b1", [128, B1_COLS], F32, kind="ExternalInput")
    b2_d = nc.dram_tensor("b2", [128, B2_COLS], F32, kind="ExternalInput")
    ya_d = nc.dram_tensor("ya", [128, YH], F32, kind="ExternalInput")
    yb_d = nc.dram_tensor("yb", [128, YH], F32, kind="ExternalInput")
    out_d = nc.dram_tensor("out", [1, 1], F32, kind="ExternalOutput")

    with tile.TileContext(nc) as tc:
        with (
            tc.tile_pool(name="sb", bufs=1) as sb,
            tc.tile_pool(name="ps", bufs=1, space="PSUM") as ps,
        ):
            # x-parts first on both queues so L1 never waits on y; the
            # y halves ride behind them.  SWDGE (gpsimd) carries blob2 so
            # the ACT ring stays free for its table load + activations.
            dum_s = sb.tile([128, 1], F32, tag="dum")
            nc.scalar.activation(out=dum_s[:], in_=nc.const_aps.aps[(F32, 0.0)],
                                 func=AF.Tanh)

            b1_s = sb.tile([128, B1_COLS], F32, tag="b1")
            nc.sync.dma_start(out=b1_s[:], in_=b1_d[:])
            b2_s = sb.tile([128, B2_COLS], F32, tag="b2")
            nc.gpsimd.dma_start(out=b2_s[:], in_=b2_d[:])
            ya_s = sb.tile([128, YH], F32, tag="ya")
            nc.sync.dma_start(out=ya_s[:], in_=ya_d[:])
            yb_s = sb.tile([128, YH], F32, tag="yb")
            nc.gpsimd.dma_start(out=yb_s[:], in_=yb_d[:])

            yA = ya_s[:]
            yB = yb_s[:]

            # PE sits idle ~3.5us while inputs stream; run dummy matmuls so
            # the HAM clock-gate is at 8/8 when the real chain starts.
            wu_s = sb.tile([128, R], F32, tag="wu")
            nc.vector.memset(wu_s[:], 0.0)
            wu_p = ps.tile([R, R], F32, tag="wup")
            for _ in range(16):
                nc.tensor.matmul(wu_p[:], wu_s[:], wu_s[:],
                                 start=True, stop=True)

            # ---- y column moments: squares on GPSIMD (idle after its DMA
            # issues), M1/S2 free-dim reduces on DVE, ACT stays clear for
            # relu/tanh/exp.
            ysq_s = sb.tile([128, YH], F32, tag="ysq")   # ya^2; 0:64 = yc^2
            ysqB_s = sb.tile([128, YH], F32, tag="ysqB")
            momh_s = sb.tile([128, 4], F32, tag="momh")
            nc.gpsimd.tensor_mul(ysq_s[:], yA, yA)
            nc.vector.tensor_reduce(out=momh_s[:, 1:2], in_=yA,
                                    axis=mybir.AxisListType.X, op=ALU.add)
            nc.vector.tensor_reduce(out=momh_s[:, 0:1], in_=ysq_s[:],
                                    axis=mybir.AxisListType.X, op=ALU.add)

            # ---- MLP layer 1, both nets in one accumulation chain ----
            hb_p = ps.tile([W1C, R], F32, tag="hb")
            xt_views = [
                b1_s[:, A_XT:A_XT + R],
                b1_s[:, A_XT + R:A_XT + 2 * R],
                b1_s[:, A_XT + 2 * R:A_XT + 3 * R],
                b2_s[:, B_XT:B_XT + R],
                b2_s[:, B_XT + R:B_XT + 2 * R],
                b2_s[:, B_XT + 2 * R:B_XT + 3 * R],
            ]
            order = [0, 1, 2, 3, 4, 5]
            for i, k in enumerate(order):
                nc.tensor.matmul(hb_p[:], b1_s[:, k * W1C:(k + 1) * W1C],
                                 xt_views[k],
                                 start=(i == 0), stop=(i == len(order) - 1))

            # ---- fused bias+relu on ACT; rows 8/40 become ones-rows ----
            hb_s = sb.tile([W1C, R], F32, tag="hbs")
            nc.scalar.activation(out=hb_s[:], in_=hb_p[:], func=AF.Relu,
                                 bias=b1_s[0:W1C, A_B1:A_B1 + 1])

            # ---- second-half moments ----
            nc.gpsimd.tensor_mul(ysqB_s[:], yB, yB)
            nc.vector.tensor_reduce(out=momh_s[:, 3:4], in_=yB,
                                    axis=mybir.AxisListType.X, op=ALU.add)
            nc.vector.tensor_reduce(out=momh_s[:, 2:3], in_=ysqB_s[:],
                                    axis=mybir.AxisListType.X, op=ALU.add)

            # ---- MLP layer 2 (bias via ones-rows): mu, z_lv in PSUM ----
            mu_p = ps.tile([Y_DIM, R], F32, tag="mup")
            lv_p = ps.tile([Y_DIM, R], F32, tag="lvp")
            nc.tensor.matmul(mu_p[:], b2_s[0:9, 0:Y_DIM], hb_s[0:9, :],
                             start=True, stop=True)
            nc.tensor.matmul(lv_p[:], b2_s[32:41, 0:Y_DIM], hb_s[32:41, :],
                             start=True, stop=True)

            # ---- inv = exp(-tanh(z_lv)) on ACT ----
            lv_s = sb.tile([Y_DIM, R], F32, tag="lvs")
            nc.scalar.activation(out=lv_s[:], in_=lv_p[:], func=AF.Tanh)
            inv_s = sb.tile([Y_DIM, R], F32, tag="invs")
            nc.scalar.activation(out=inv_s[:], in_=lv_s[:], func=AF.Exp,
                                 scale=-1.0)

            # ---- combine half-moments; G = yc*B - M1 ; K from ysq ----
            s2c_s = sb.tile([128, 1], F32, tag="s2c")
            nc.vector.tensor_scalar(out=s2c_s[:], in0=momh_s[:, 0:1],
                                    scalar1=momh_s[:, 2:3],
                                    scalar2=0.5 / (B * B),
                                    op0=ALU.add, op1=ALU.mult)
            m1_s = sb.tile([128, 1], F32, tag="m1")
            nc.vector.tensor_scalar(out=m1_s[:], in0=momh_s[:, 1:2],
                                    scalar1=momh_s[:, 3:4],
                                    scalar2=1.0 / (B * B),
                                    op0=ALU.add, op1=ALU.mult)

            g_s = sb.tile([128, R], F32, tag="gs")
            nc.vector.tensor_scalar(out=g_s[:], in0=ya_s[:, 0:R],
                                    scalar1=1.0 / B, scalar2=m1_s[:],
                                    op0=ALU.mult, op1=ALU.subtract)
            k_s = sb.tile([128, R], F32, tag="ks")
            nc.vector.tensor_scalar(out=k_s[:], in0=ysq_s[:, 0:R],
                                    scalar1=-0.5 / B, scalar2=s2c_s[:],
                                    op0=ALU.mult, op1=ALU.add)

            # ---- tail: t = mu*G ; q = t/B^2 + K ; w = q*inv ; reduce ----
            t_s = sb.tile([Y_DIM, R], F32, tag="ts")
            nc.vector.tensor_mul(t_s[:], mu_p[:], g_s[:])
            q_s = sb.tile([Y_DIM, R], F32, tag="qs")
            nc.vector.tensor_add(q_s[:], t_s[:], k_s[:])
            w_s = sb.tile([Y_DIM, R], F32, tag="ws")
            nc.vector.tensor_mul(w_s[:], q_s[:], inv_s[:])
            tot_s = sb.tile([128, 1], F32, tag="tot")
            nc.vector.tensor_reduce(out=tot_s[:], in_=w_s[:],
                                    axis=mybir.AxisListType.X, op=ALU.add)

            # ---- cross-partition reduce on PE -> [1,1] -> 4B DMA out ----
            ones_ap = nc.const_aps.aps[(F32, 1.0)]
            res_p = ps.tile([1, 1], F32, tag="res")
            nc.tensor.matmul(res_p[:], tot_s[:], ones_ap,
                             start=True, stop=True)
            res_s = sb.tile([1, 1], F32, tag="ress")
            nc.vector.tensor_copy(out=res_s[:], in_=res_p[:])
            nc.sync.dma_start(out=out_d[:], in_=res_s[:])

    nc.compile()
    return nc


def _get_nc():
    if "nc" not in _CACHE:
        _CACHE["nc"] = _build()
    return _CACHE["nc"]


def _pack_weights(w1_mu, b1_mu, w2_mu, b2_mu, w1_lv, b1_lv, w2_lv, b2_lv):
    f = np.float32
    wa = np.zeros((128, A_XT), f)
    w1m = np.asarray(w1_mu, f).reshape(XC, 128, HID)
    w1l = np.asarray(w1_lv, f).reshape(XC, 128, HID)
    for k in range(XC):
        wa[:, k * W1C:k * W1C + 8] = w1m[k]
        wa[:, k * W1C + 32:k * W1C + 40] = w1l[k]
    wa[0:8, A_B1] = np.asarray(b1_mu, f)
    wa[8, A_B1] = 1.0
    wa[32:40, A_B1] = np.asarray(b1_lv, f)
    wa[40, A_B1] = 1.0
    wb = np.zeros((128, Y_DIM), f)
    wb[0:8, :] = np.asarray(w2_mu, f)
    wb[8, :] = np.asarray(b2_mu, f)
    wb[32:40, :] = np.asarray(w2_lv, f)
    wb[40, :] = np.asarray(b2_lv, f)
    return wa, wb


def kernel(x_samples, y_samples, w1_mu, b1_mu, w2_mu, b2_mu,
           w1_lv, b1_lv, w2_lv, b2_lv, **profile_kwargs):
    from concourse import bass_utils

    f = np.float32
    wa, wb = _pack_weights(w1_mu, b1_mu, w2_mu, b2_mu,
                           w1_lv, b1_lv, w2_lv, b2_lv)
    yt = np.ascontiguousarray(np.asarray(y_samples, f).T)      # [128, 512]
    x = np.asarray(x_samples, f)

    in_maps = []
    for c in range(N_CORES):
        xt = np.ascontiguousarray(x[c * R:(c + 1) * R].T).reshape(XC, 128, R)
        ytc = np.roll(yt, -c * R, axis=1)
        b1 = np.empty((128, B1_COLS), f)
        b1[:, :A_XT] = wa
        for k in range(3):
            b1[:, A_XT + k * R:A_XT + (k + 1) * R] = xt[k]
        b2 = np.empty((128, B2_COLS), f)
        b2[:, :B_W2] = wb
        for k in range(3):
            b2[:, B_XT + k * R:B_XT + (k + 1) * R] = xt[3 + k]
        in_maps.append({"b1": b1, "b2": b2,
                        "ya": np.ascontiguousarray(ytc[:, :YH]),
                        "yb": np.ascontiguousarray(ytc[:, YH:])})

    nc = _get_nc()
    res = bass_utils.run_bass_kernel_spmd(
        nc, in_maps, core_ids=list(range(N_CORES)), **profile_kwargs
    )
    total = sum(float(m["out"][0, 0]) for m in res.results)
    total -= np.log1p(np.exp(-20.0) / (B - 1))
    out = np.array(total, dtype=np.float32)
    if profile_kwargs:
        return out, res
    return out


# revision 18
# speedup vs baseline: 1.2027x; 1.2027x over previous
"""Trainium2 Bass kernel for nn_L1OutUB (L1-out upper bound contrastive loss).

Math: the reference builds a [B,B,B] tensor `inpt[a,i,j] = all_probs[i,j] +
(-20 if a==i else 0)` and logsumexps over `a`.  That logsumexp is exactly
`all_probs[i,j] + log(B-1+e^-20)`, so

    result = mean(positive) - mean(all_probs) - log1p(e^-20 / (B-1))

and `sum_j all_probs[i,j]` collapses onto per-column moments of y
(S2[d] = sum_j y[j,d]^2, M1[d] = sum_j y[j,d]).  The -0.5*logvar terms
cancel exactly between positive and negative, and the per-(i,d) mu^2 terms
cancel between the positive and all-pairs branches:

    contrib[i,d] = inv[i,d] * ( mu[i,d]*(yc/B - M1/B^2) + K[i,d] )
      K   = S2/(2B^2) - yc^2/(2B)     (yc = matched y rows, feature-major)
      inv = exp(-tanh(z_lv))

Sharding: rows of x across 8 cores (64 rows each); every core gets the full
y (column-rotated so its matched rows sit at cols 0:64 of yT) and computes
the global column moments redundantly.  Host sums the 8 scalar partials
(the "all-reduce").

Layout/overlap decisions (all transposes done on host; PE does matmuls only):
  - Two input DMAs per HWDGE queue: blob1 = [w1|b1|xT chunks 0:2|yT half A],
    blob2 = [w2|xT chunks 3:5|yT half B].  x parts stream ahead of y parts;
    y moments are computed per-half as the data lands.
  - y is shipped pre-transposed (yT [128,512]) so moments are free-dim DVE
    reductions and yc/yc^2 are column slices.
  - L1 runs both nets in one 6-matmul chain ([128,41] stationaries, mu rows
    0:8, lv rows 32:40).  Bias+relu fused into one ACT op whose bias column
    also manufactures the two all-ones rows (bias[8]=bias[40]=1, relu(0+1)).
  - L2 folds its biases via those ones-rows, so mu / z_lv leave PSUM done.
  - ACT does relu/tanh/exp only (one table set, load overlaps the DMAs).
  - Final reduce: free-dim DVE reduce -> [128,1], PE matmul against a ones
    column -> [1,1] -> single 4-byte output DMA (a [128,1] output DMA costs
    ~7us in scattered-write completion; don't do that).
"""

import numpy as np

import concourse.bacc as bacc
import concourse.tile as tile
from concourse import mybir

F32 = mybir.dt.float32
AF = mybir.ActivationFunctionType
ALU = mybir.AluOpType

B, X_DIM, Y_DIM, HID = 512, 768, 128, 8
N_CORES = 8
R = B // N_CORES          # rows per core = 64
XC = X_DIM // 128         # x feature chunks = 6
YH = B // 2               # yT half width = 256

W1C = 41                  # L1 stationary cols (mu 0:8, lv 32:40, 40 = ones)
A_W1 = XC * W1C           # 246
A_B1 = A_W1               # bias column index
A_XT = A_B1 + 1           # 247
B1_COLS = A_XT + 3 * R    # blob1 width: 439 (w1|b1|xT chunks 0:2)
B_W2 = Y_DIM              # blob2: w2 block cols 0:128
B_XT = B_W2               # xT chunks 3:5 at 128:320
B2_COLS = B_XT + 3 * R    # blob2 width: 320

_CACHE = {}


def _build():
    nc = bacc.Bacc("TRN2", target_bir_lowering=False, debug=False,
                   num_devices=N_CORES)

    b1_d = nc.dram_tensor("b1", [128, B1_COLS], F32, kind="ExternalInput")
    b2_d = nc.dram_tensor("b2", [128, B2_COLS], F32, kind="ExternalInput")
    ya_d = nc.dram_tensor("ya", [128, YH], F32, kind="ExternalInput")
    yb_d = nc.dram_tensor("yb", [128, YH], F32, kind="ExternalInput")
    out_d = nc.dram_tensor("out", [1, 1], F32, kind="ExternalOutput")

    with tile.TileContext(nc) as tc:
        with (
            tc.tile_pool(name="sb", bufs=1) as sb,
            tc.tile_pool(name="ps", bufs=1, space="PSUM") as ps,
        ):
            # x-parts first on both queues so L1 never waits on y; the
            # y halves ride behind them.  SWDGE (gpsimd) carries blob2 so
            # the ACT ring stays free for its table load + activations.
            dum_s = sb.tile([128, 1], F32, tag="dum")
            nc.scalar.activation(out=dum_s[:], in_=nc.const_aps.aps[(F32, 0.0)],
                                 func=AF.Tanh)

            b1_s = sb.tile([128, B1_COLS], F32, tag="b1")
            nc.sync.dma_start(out=b1_s[:], in_=b1_d[:])
            b2_s = sb.tile([128, B2_COLS], F32, tag="b2")
            nc.gpsimd.dma_start(out=b2_s[:], in_=b2_d[:])
            ya_s = sb.tile([128, YH], F32, tag="ya")
            nc.sync.dma_start(out=ya_s[:], in_=ya_d[:])
            yb_s = sb.tile([128, YH], F32, tag="yb")
            nc.gpsimd.dma_start(out=yb_s[:], in_=yb_d[:])

            yA = ya_s[:]
            yB = yb_s[:]

            # PE sits idle ~3.5us while inputs stream; run dummy matmuls so
            # the HAM clock-gate is at 8/8 when the real chain starts.
            wu_s = sb.tile([128, 128], F32, tag="wu")
            nc.vector.memset(wu_s[:], 0.0)
            wu_p = ps.tile([128, 128], F32, tag="wup")
            for _ in range(8):
                nc.tensor.matmul(wu_p[:], wu_s[:], wu_s[:],
                                 start=True, stop=True)

            # ---- y column moments: half A now (square+S2 fused on ACT,
            # M1 on DVE); half B is emitted after relu so the relu slot on
            # ACT isn't blocked behind it.
            ysq_s = sb.tile([128, YH], F32, tag="ysq")   # ya^2; 0:64 = yc^2
            ysqB_s = sb.tile([128, YH], F32, tag="ysqB")
            momh_s = sb.tile([128, 4], F32, tag="momh")
            nc.vector.tensor_reduce(out=momh_s[:, 1:2], in_=yA,
                                    axis=mybir.AxisListType.X, op=ALU.add)
            nc.scalar.activation(out=ysq_s[:], in_=yA, func=AF.Square,
                                 accum_out=momh_s[:, 0:1])

            # ---- MLP layer 1, both nets in one accumulation chain ----
            hb_p = ps.tile([W1C, R], F32, tag="hb")
            xt_views = [
                b1_s[:, A_XT:A_XT + R],
                b1_s[:, A_XT + R:A_XT + 2 * R],
                b1_s[:, A_XT + 2 * R:A_XT + 3 * R],
                b2_s[:, B_XT:B_XT + R],
                b2_s[:, B_XT + R:B_XT + 2 * R],
                b2_s[:, B_XT + 2 * R:B_XT + 3 * R],
            ]
            order = [0, 1, 2, 3, 4, 5]
            for i, k in enumerate(order):
                nc.tensor.matmul(hb_p[:], b1_s[:, k * W1C:(k + 1) * W1C],
                                 xt_views[k],
                                 start=(i == 0), stop=(i == len(order) - 1))

            # ---- fused bias+relu on ACT; rows 8/40 become ones-rows ----
            hb_s = sb.tile([W1C, R], F32, tag="hbs")
            nc.scalar.activation(out=hb_s[:], in_=hb_p[:], func=AF.Relu,
                                 bias=b1_s[0:W1C, A_B1:A_B1 + 1])

            # ---- second-half moments (ACT slot right after relu) ----
            nc.vector.tensor_reduce(out=momh_s[:, 3:4], in_=yB,
                                    axis=mybir.AxisListType.X, op=ALU.add)
            nc.scalar.activation(out=ysqB_s[:], in_=yB, func=AF.Square,
                                 accum_out=momh_s[:, 2:3])

            # ---- MLP layer 2 (bias via ones-rows): mu, z_lv in PSUM ----
            mu_p = ps.tile([Y_DIM, R], F32, tag="mup")
            lv_p = ps.tile([Y_DIM, R], F32, tag="lvp")
            nc.tensor.matmul(mu_p[:], b2_s[0:9, 0:Y_DIM], hb_s[0:9, :],
                             start=True, stop=True)
            nc.tensor.matmul(lv_p[:], b2_s[32:41, 0:Y_DIM], hb_s[32:41, :],
                             start=True, stop=True)

            # ---- inv = exp(-tanh(z_lv)) on ACT ----
            lv_s = sb.tile([Y_DIM, R], F32, tag="lvs")
            nc.scalar.activation(out=lv_s[:], in_=lv_p[:], func=AF.Tanh)
            inv_s = sb.tile([Y_DIM, R], F32, tag="invs")
            nc.scalar.activation(out=inv_s[:], in_=lv_s[:], func=AF.Exp,
                                 scale=-1.0)

            # ---- combine half-moments; G = yc*B - M1 ; K from ysq ----
            s2c_s = sb.tile([128, 1], F32, tag="s2c")
            nc.vector.tensor_scalar(out=s2c_s[:], in0=momh_s[:, 0:1],
                                    scalar1=momh_s[:, 2:3],
                                    scalar2=0.5 / (B * B),
                                    op0=ALU.add, op1=ALU.mult)
            m1_s = sb.tile([128, 1], F32, tag="m1")
            nc.vector.tensor_scalar(out=m1_s[:], in0=momh_s[:, 1:2],
                                    scalar1=momh_s[:, 3:4],
                                    scalar2=1.0 / (B * B),
                                    op0=ALU.add, op1=ALU.mult)

            g_s = sb.tile([128, R], F32, tag="gs")
            nc.vector.tensor_scalar(out=g_s[:], in0=ya_s[:, 0:R],
                                    scalar1=1.0 / B, scalar2=m1_s[:],
                                    op0=ALU.mult, op1=ALU.subtract)
            k_s = sb.tile([128, R], F32, tag="ks")
            nc.vector.tensor_scalar(out=k_s[:], in0=ysq_s[:, 0:R],
                                    scalar1=-0.5 / B, scalar2=s2c_s[:],
                                    op0=ALU.mult, op1=ALU.add)

            # ---- tail: t = mu*G ; q = t/B^2 + K ; w = q*inv ; reduce ----
            t_s = sb.tile([Y_DIM, R], F32, tag="ts")
            nc.vector.tensor_mul(t_s[:], mu_p[:], g_s[:])
            q_s = sb.tile([Y_DIM, R], F32, tag="qs")
            nc.vector.tensor_add(q_s[:], t_s[:], k_s[:])
            w_s = sb.tile([Y_DIM, R], F32, tag="ws")
            nc.vector.tensor_mul(w_s[:], q_s[:], inv_s[:])
            tot_s = sb.tile([128, 1], F32, tag="tot")
            nc.vector.tensor_reduce(out=tot_s[:], in_=w_s[:],
                                    axis=mybir.AxisListType.X, op=ALU.add)

            # ---- cross-partition reduce on PE -> [1,1] -> 4B DMA out ----
            ones_ap = nc.const_aps.aps[(F32, 1.0)]
            res_p = ps.tile([1, 1], F32, tag="res")
            nc.tensor.matmul(res_p[:], tot_s[:], ones_ap,
                             start=True, stop=True)
            res_s = sb.tile([1, 1], F32, tag="ress")
            nc.vector.tensor_copy(out=res_s[:], in_=res_p[:])
            nc.sync.dma_start(out=out_d[:], in_=res_s[:])

    nc.compile()
    return nc


def _get_nc():
    if "nc" not in _CACHE:
        _CACHE["nc"] = _build()
    return _CACHE["nc"]


def _pack_weights(w1_mu, b1_mu, w2_mu, b2_mu, w1_lv, b1_lv, w2_lv, b2_lv):
    f = np.float32
    wa = np.zeros((128, A_XT), f)
    w1m = np.asarray(w1_mu, f).reshape(XC, 128, HID)
    w1l = np.asarray(w1_lv, f).reshape(XC, 128, HID)
    for k in range(XC):
        wa[:, k * W1C:k * W1C + 8] = w1m[k]
        wa[:, k * W1C + 32:k * W1C + 40] = w1l[k]
    wa[0:8, A_B1] = np.asarray(b1_mu, f)
    wa[8, A_B1] = 1.0
    wa[32:40, A_B1] = np.asarray(b1_lv, f)
    wa[40, A_B1] = 1.0
    wb = np.zeros((128, Y_DIM), f)
    wb[0:8, :] = np.asarray(w2_mu, f)
    wb[8, :] = np.asarray(b2_mu, f)
    wb[32:40, :] = np.asarray(w2_lv, f)
    wb[40, :] = np.asarray(b2_lv, f)
    return wa, wb


def kernel(x_samples, y_samples, w1_mu, b1_mu, w2_mu, b2_mu,
           w1_lv, b1_lv, w2_lv, b2_lv, **profile_kwargs):
    from concourse import bass_utils

    f = np.float32
    wa, wb = _pack_weights(w1_mu, b1_mu, w2_mu, b2_mu,
                           w1_lv, b1_lv, w2_lv, b2_lv)
    yt = np.ascontiguousarray(np.asarray(y_samples, f).T)      # [128, 512]
    x = np.asarray(x_samples, f)

    in_maps = []
    for c in range(N_CORES):
        xt = np.ascontiguousarray(x[c * R:(c + 1) * R].T).reshape(XC, 128, R)
        ytc = np.roll(yt, -c * R, axis=1)
        b1 = np.empty((128, B1_COLS), f)
        b1[:, :A_XT] = wa
        for k in range(3):
            b1[:, A_XT + k * R:A_XT + (k + 1) * R] = xt[k]
        b2 = np.empty((128, B2_COLS), f)
        b2[:, :B_W2] = wb
        for k in range(3):
            b2[:, B_XT + k * R:B_XT + (k + 1) * R] = xt[3 + k]
        in_maps.append({"b1": b1, "b2": b2,
                        "ya": np.ascontiguousarray(ytc[:, :YH]),
                        "yb": np.ascontiguousarray(ytc[:, YH:])})

    nc = _get_nc()
    res = bass_utils.run_bass_kernel_spmd(
        nc, in_maps, core_ids=list(range(N_CORES)), **profile_kwargs
    )
    total = sum(float(m["out"][0, 0]) for m in res.results)
    total -= np.log1p(np.exp(-20.0) / (B - 1))
    out = np.array(total, dtype=np.float32)
    if profile_kwargs:
        return out, res
    return out
